# revision 1
# baseline (speedup 1.0000x reference)
"""Trainium2 Bass kernel for DigitConvolutionalModel:
    out = relu(conv2d_3x3_valid(x.reshape(B,28,28))) .reshape(B,676) @ W + b

Strategy (pure data parallel over 8 cores, B=32768 -> 4096/core), v3:

Per core, samples are processed in 8 groups of N=512. The marshaled x
puts one sample's row-group t (4 image rows x 28 cols = 112 pixels) on
112 partitions, with free dim (t, n):
    x_d[g, 28*rl + c, 512*t + n] = x[b(g,n), 28*(4t+rl) + c]
so a group loads with ONE contiguous DMA (7168 B per partition).

Conv: output chunk t = output rows 4t..4t+3 (t=6: rows 24..25), M =
26*il + c padded to 128. Chunk t needs input rows 4t..4t+5: rows
4t..4t+3 come from row-group t (112-partition matmul), rows 4t+4/4t+5
from the first 56 partitions of row-group t+1 (PSUM-accumulated spill
matmul). 13 matmuls of N=512 per group (vs 21 in v1), DMA stays minimal
(6.4 MB fp16 per core).

Relu evacuates each conv chunk's PSUM [<=104, 512] into fp16 h_t tiles
(partitions = features 104t..104t+103), alternating DVE/ACT.

FC is 4-way column-tiled: groups run in half-batches of 4; one
psf[128, 512] holds the 4 groups' outputs in 32-partition strips.  A
single K=1 matmul (bias4 x ones) seeds the bias, then each chunk t
issues 4 strip matmuls (tile_position=(0, 32*gg)) that execute
concurrently in the PE array's four column groups.

Tail: ACT copies psf -> SBUF fp32, DVE 32x32-block transposes + gather
copies collect everything into outsb[32, 1280]; sample mapping
b = 128*p + 16*g + X makes the single final store DMA fully contiguous
(5120 B runs per partition).
"""

import sys
import numpy as np

for _p in ("/opt/trn_rl_repo", "/root/.axon_site/_ro/trn_rl_repo"):
    if _p not in sys.path:
        sys.path.insert(0, _p)

import concourse.bass as bass  # noqa: E402,F401
import concourse.tile as tile  # noqa: E402
from concourse import bacc, mybir  # noqa: E402
from concourse.bass_utils import run_bass_kernel_spmd  # noqa: E402

IMG = 28
KW = 3
OUT = 26  # IMG - KW + 1
NPIX = IMG * IMG          # 784
NOUTPIX = OUT * OUT       # 676
NCLS = 10
NCORES = 8
B_TOTAL = 32768
B_CORE = B_TOTAL // NCORES   # 4096
NG = 8                       # groups per core
N = 512                      # samples per group
NT = 7                       # row-groups (4 rows x 28 cols = 112 partitions)
CH = 104                     # features per conv chunk (4 out rows x 26)
F32 = mybir.dt.float32
F16 = mybir.dt.float16

_CACHE = {}


def _chunk_m(t):
    """Valid output rows (M) of chunk t: 104 for t<6, 52 for t=6."""
    return 52 if t == NT - 1 else 104


def _build_program(mm_dtype=F16, hwloop=0, stage=5, internal_x=False,
                   fc_coltile=True, dma_groups=2, predma=False,
                   dma_dummy=False, conv_variant="normal", dma_eng="hw"):
    """Build + compile the per-core Bass program (identical on all cores)."""
    nc = bacc.Bacc("TRN2", target_bir_lowering=False, debug=False,
                   num_devices=NCORES)

    x_d = nc.dram_tensor("x", (NG, 112, NT * N), mm_dtype,
                         kind="Internal" if internal_x else "ExternalInput")
    main_d = nc.dram_tensor("mainT", (112, NT * 128), mm_dtype,
                            kind="ExternalInput")
    spill_d = nc.dram_tensor("spillT", (56, (NT - 1) * 128), mm_dtype,
                             kind="ExternalInput")
    wsb_d = nc.dram_tensor("wsbT", (CH, NT * 32), mm_dtype,
                           kind="ExternalInput")
    bias_d = nc.dram_tensor("biasv", (1, 128), mm_dtype, kind="ExternalInput")
    ones_d = nc.dram_tensor("ones", (1, N), mm_dtype, kind="ExternalInput")
    out_d = nc.dram_tensor("out", (B_CORE, NCLS), F32, kind="ExternalOutput")

    x_ap = x_d.ap()
    out_ap = out_d.ap()
    GPB = 4 if fc_coltile else 1          # groups per FC batch

    with tile.TileContext(nc) as tc:
        with (
            tc.tile_pool(name="consts", bufs=1) as consts,
            tc.tile_pool(name="xin", bufs=max(2, NG // dma_groups)) as xin,
            tc.tile_pool(name="hbuf", bufs=2) as hbuf,
            tc.tile_pool(name="obuf", bufs=2) as obuf,
            tc.tile_pool(name="convps", bufs=6, space="PSUM") as convps,
            tc.tile_pool(name="fcps", bufs=2, space="PSUM") as fcps,
        ):
            mainT = consts.tile([112, NT * 128], mm_dtype)
            spillT = consts.tile([56, (NT - 1) * 128], mm_dtype)
            wsbT = consts.tile([CH, NT * 32], mm_dtype)
            biasv = consts.tile([1, 128], mm_dtype)
            ones = consts.tile([1, N], mm_dtype)
            nc.sync.dma_start(out=mainT[:, :], in_=main_d.ap())
            nc.sync.dma_start(out=spillT[:, :], in_=spill_d.ap())
            nc.sync.dma_start(out=wsbT[:, :], in_=wsb_d.ap())
            nc.sync.dma_start(out=biasv[:, :], in_=bias_d.ap())
            nc.sync.dma_start(out=ones[:, :], in_=ones_d.ap())

            xpre = {}
            if predma:
                for g in range(NG):
                    xp = consts.tile([112, NT * N], mm_dtype,
                                     name=f"xp{g}")
                    nc.sync.dma_start(out=xp[:, :], in_=x_ap[g])
                    xpre[g] = (xp, 0)

            import contextlib
            loop_cm = (tc.For_i(0, hwloop, 1) if hwloop
                       else contextlib.nullcontext())
            with loop_cm:
                outsb = obuf.tile([32, NG * 16 * NCLS], F32, tag="outsb")
                xts = {}
                for batch in range(NG // GPB):
                    h_all = []
                    for gg in range(GPB):
                        g = batch * GPB + gg
                        # ---- load dma_groups groups per contiguous DMA ----
                        if predma:
                            xts[g] = xpre[g]
                            if dma_dummy:
                                xd = xin.tile([112, NT * N], mm_dtype,
                                              tag="xd")
                                nc.sync.dma_start(out=xd[:, :],
                                                  in_=x_ap[g])
                        elif g % dma_groups == 0:
                            xt = xin.tile([112, dma_groups * NT * N],
                                          mm_dtype, tag="xt")
                            if dma_eng == "gpsimd":
                                nc.gpsimd.dma_start(
                                    out=xt[:, :].rearrange(
                                        "q (g n) -> q g n", g=dma_groups),
                                    in_=x_ap[g:g + dma_groups].rearrange(
                                        "g q n -> q g n"))
                            elif dma_eng == "split":
                                # halves of the free dim on both HWDGE rings
                                H = NT * N // 2
                                for si, eng in enumerate(
                                        (nc.sync, nc.scalar)):
                                    nc_ = eng.dma_start(
                                        out=xt[:, :].rearrange(
                                            "q (g n) -> q g n",
                                            g=dma_groups)[:, :,
                                                          si * H:
                                                          (si + 1) * H],
                                        in_=x_ap[g:g + dma_groups, :,
                                                 si * H:(si + 1) * H]
                                        .rearrange("g q n -> q g n"))
                            else:
                                eng = (nc.sync if (g // dma_groups) % 2
                                       == 0 else nc.scalar)
                                eng.dma_start(
                                    out=xt[:, :].rearrange(
                                        "q (g n) -> q g n", g=dma_groups),
                                    in_=x_ap[g:g + dma_groups].rearrange(
                                        "g q n -> q g n"))
                            for k in range(dma_groups):
                                xts[g + k] = (xt, k * NT * N)
                        xtile, xoff = xts[g]
                        if stage < 1:
                            continue
                        if stage < 2:
                            dmy = obuf.tile([32, 8], F32, tag="dmy")
                            nc.vector.tensor_copy(
                                dmy[0:1, 0:8],
                                xtile[0:1, xoff:xoff + 16].bitcast(F32))
                            continue
                        xv = xtile[:, xoff:xoff + NT * N].rearrange(
                            "q (t n) -> q t n", n=N)

                        # ---- conv: 13 matmuls (7 main + 6 spill) ----
                        # Issue order m0 m1 s0 m2 s1 ... m6 s5 so each
                        # accumulating spill lands >=2 matmuls after its
                        # main: the same-bank PSUM drain-wait is hidden
                        # behind an independent matmul.
                        hts = {}
                        pqs = {}

                        def relu_t(t):
                            m = _chunk_m(t)
                            ht = hbuf.tile([CH, N], mm_dtype,
                                           tag=f"h{gg}_{t}")
                            if t % 2 == 0:
                                nc.vector.tensor_scalar_max(
                                    ht[0:m, :], pqs[t][0:m, 0:N], 0.0)
                            else:
                                nc.scalar.activation(
                                    ht[0:m, :], pqs[t][0:m, 0:N],
                                    mybir.ActivationFunctionType.Relu)
                            hts[t] = ht

                        order = [("m", 0)]
                        for t in range(1, NT):
                            order += [("m", t), ("s", t - 1)]
                        mains_only = conv_variant == "mains"
                        if conv_variant == "serial":
                            order = []
                            for t in range(NT):
                                order.append(("m", t))
                                if t < NT - 1:
                                    order.append(("s", t))
                        for kind, t in order:
                            mt = 0 if conv_variant == "samelhs" else t
                            if kind == "m":
                                pq = convps.tile([128, N], F32, tag="pq")
                                pqs[t] = pq
                                nc.tensor.matmul(
                                    pq[0:128, 0:N],
                                    mainT[0:112, 128 * mt:128 * mt + 128],
                                    xv[:, t, :],
                                    start=True,
                                    stop=(t == NT - 1 or mains_only),
                                )
                                if stage >= 3 and (t == NT - 1
                                                   or mains_only):
                                    relu_t(t)
                            elif not mains_only:
                                nc.tensor.matmul(
                                    pqs[t][0:128, 0:N],
                                    spillT[0:56, 128 * mt:128 * mt + 128],
                                    xv[0:56, t + 1, :],
                                    start=False, stop=True,
                                )
                                if stage >= 3:
                                    relu_t(t)
                        h_all.append([hts[t] for t in sorted(hts)]
                                     if stage >= 3 else [])

                    if stage < 4:
                        continue
                    # ---- FC: bias mm + col-tiled strip matmuls ----
                    psf = fcps.tile([32 * GPB, N], F32, tag="psf")
                    nc.tensor.matmul(psf[0:32 * GPB, 0:N],
                                     biasv[0:1, 0:32 * GPB],
                                     ones[0:1, :], start=True, stop=False)
                    for t in range(NT):
                        m = _chunk_m(t)
                        for gg in range(GPB):
                            nc.tensor.matmul(
                                psf[32 * gg:32 * gg + 32, 0:N],
                                wsbT[0:m, 32 * t:32 * t + 32],
                                h_all[gg][t][0:m, :],
                                start=False, stop=(t == NT - 1),
                                tile_position=(0, 32 * gg) if fc_coltile
                                else None,
                            )
                    if stage < 5:
                        continue
                    # ---- tail: evac, 32x32 transposes, gather ----
                    osb = obuf.tile([32 * GPB, N], F32, tag="osb")
                    nc.scalar.copy(osb[:, :], psf[0:32 * GPB, 0:N])
                    tt = obuf.tile([32 * GPB, N], F32, tag="tt")
                    for kk in range(4):
                        nc.vector.transpose(
                            tt[0:32 * GPB, 128 * kk:128 * kk + 128],
                            osb[0:32 * GPB, 128 * kk:128 * kk + 128])
                    # outsb[p, 160g + 10X + o] = tt[32gg + p, 32X + o]
                    for gg in range(GPB):
                        g = batch * GPB + gg
                        nc.vector.tensor_copy(
                            outsb[:, 160 * g:160 * (g + 1)].rearrange(
                                "p (X o) -> p X o", o=NCLS),
                            tt[32 * gg:32 * gg + 32, :].rearrange(
                                "p (X o) -> p X o", o=32)[:, :, 0:NCLS])
                if stage >= 5:
                    # b = 128*p + 16*g + X  ->  one contiguous store
                    nc.sync.dma_start(
                        out=out_ap[:, :].rearrange("(p r) o -> p r o", p=32),
                        in_=outsb[:, :].rearrange("p (r o) -> p r o",
                                                  o=NCLS))

    nc.compile()
    return nc


def _host_constants(conv_w, W, b):
    """Conv chunk lhsTs (main + spill), FC chunk lhsTs, bias row."""
    mainT = np.zeros((112, NT * 128), np.float32)
    spillT = np.zeros((56, (NT - 1) * 128), np.float32)
    for t in range(NT):
        nil = 4 if t < NT - 1 else 2
        for il in range(nil):
            for c in range(OUT):
                mcol = 26 * il + c
                for rl in range(4):
                    di = rl - il
                    if not (0 <= di < KW):
                        continue
                    for cq in range(c, c + KW):
                        mainT[28 * rl + cq, 128 * t + mcol] = \
                            conv_w[di, cq - c]
                if t < NT - 1:
                    for rl2 in range(2):
                        di = 4 + rl2 - il
                        if not (0 <= di < KW):
                            continue
                        for cq in range(c, c + KW):
                            spillT[28 * rl2 + cq, 128 * t + mcol] = \
                                conv_w[di, cq - c]
    wsbT = np.zeros((CH, NT * 32), np.float32)
    for t in range(NT):
        m = _chunk_m(t)
        wsbT[0:m, 32 * t:32 * t + NCLS] = W[CH * t:CH * t + m, :]
    biasv = np.zeros((1, 128), np.float32)
    for gg in range(4):
        biasv[0, 32 * gg:32 * gg + NCLS] = b
    ones = np.ones((1, N), np.float32)
    return mainT, spillT, wsbT, biasv, ones


def _marshal_x(x):
    """[B, 784] fp32 -> per-core [NG, 112, 7*512] fp16.

    Column n (= 32*X + p, p<32, X<16) of group g holds sample
    b = 4096*core + 128*p + 16*g + X, so the final store
    (b = 128*p + r, r = 16*g + X) is one contiguous DMA.
    x_d[g, 28*rl + c, 512*t + n] = x[b, 28*(4t+rl) + c].
    """
    xs = x.reshape(NCORES, 32, NG, 16, NT, 4, IMG)  # core p g X t rl c
    xs = xs.transpose(0, 2, 5, 6, 4, 3, 1)          # core g rl c t X p
    xs = np.ascontiguousarray(xs, dtype=np.float16)
    return xs.reshape(NCORES, NG, 112, NT * N)


def _unmarshal_out(res):
    """Per-core out [4096, 10] is already in natural sample order."""
    return np.concatenate(res, axis=0)


def _run(x, conv_w, W, b, trace=False, mm_dtype=F16):
    x = np.ascontiguousarray(np.asarray(x, dtype=np.float32))
    conv_w = np.asarray(conv_w, dtype=np.float32)
    W = np.asarray(W, dtype=np.float32)
    b = np.asarray(b, dtype=np.float32)
    assert x.shape == (B_TOTAL, NPIX), x.shape

    key = ("prog", str(mm_dtype))
    if key not in _CACHE:
        _CACHE[key] = _build_program(mm_dtype)
    nc = _CACHE[key]

    mainT, spillT, wsbT, biasv, ones = _host_constants(conv_w, W, b)
    xm = _marshal_x(x)
    in_maps = []
    for i in range(NCORES):
        in_maps.append({
            "x": xm[i],
            "mainT": mainT.astype(np.float16),
            "spillT": spillT.astype(np.float16),
            "wsbT": wsbT.astype(np.float16),
            "biasv": biasv.astype(np.float16),
            "ones": ones.astype(np.float16),
        })
    res = run_bass_kernel_spmd(nc, in_maps, core_ids=list(range(NCORES)),
                               trace=trace)
    out = _unmarshal_out([res.results[i]["out"] for i in range(NCORES)])
    return out, res


def kernel(x, conv_w, W, b):
    out, _ = _run(x, conv_w, W, b, trace=False)
    return out

